# revision 34
# baseline (speedup 1.0000x reference)
"""Multi-head attention (b=2, n=2048, 16 heads x 64, RoPE) on 8 TRN2 NeuronCores.

Sharding: core = 4*b + g handles batch b (0..1) and head-group g (0..3, i.e.
heads 4g..4g+3).  Each core computes its partial output projection
out_partial[b] = O_g @ Wo[256g:256g+256, :]; the host sums the 4 partials per
batch and adds the bias.

Device layout (per core, everything transposed so the contraction dim sits on
SBUF partitions):
  xT   [1024, 2048]  x[b].T  bf16                  (host pre-transposed)
  wq/wqs/wk/wks/wv [1024, 256] bf16, wo [256, 1024] f32r
  cosT/sinT [128, 2048] f32                        (RoPE tables, head-pair rows)
Pipeline: QK projections (+RoPE-swapped twins, bf16) -> RoPE combine on DVE
(fp32 psum x fp32 trig -> bf16) -> V projection -> attention per nq-chunk and
head-pair: S^T = K Q^T on PE (row-group concurrent head pairs, bf16), exp on
ACT (fp32 psum -> bf16), O'^T = [V|1]^T P^T (bf16 in, fp32 accum; ones column
yields softmax denominators) -> normalize (fp32) -> f32r output projection,
emitted per nq chunk so it overlaps the next chunk's attention.
"""

import numpy as np

HEADS = 16
DH = 64
THETA = 10000.0
B, N, DIM = 2, 2048, 1024
GCOLS = 4 * DH  # 256 columns per head-group
P = 128
NQC = 512  # nq chunk (psum bank)
NCH = N // NQC  # 4
KT = DIM // P  # 8 contraction subtiles for projections
NKT = N // P  # 16 nk tiles

_cache = {}


def _rope_tables():
    """cosT/sinT [128, 2048] in transposed (d, n) layout, head-pair rows.
    Replicates reference fp32 arithmetic."""
    d = np.float32(DH)
    inv_freq = np.float32(1.0) / (
        np.float32(THETA) ** (np.arange(0, DH, 2, dtype=np.float32) / d)
    )  # [32]
    ang = np.arange(N, dtype=np.float32)[:, None] * inv_freq[None, :]  # [n, 32] fp32
    ang = np.repeat(ang, 2, axis=-1)  # [n, 64]
    cos = np.cos(ang).astype(np.float32).T  # [64, n]
    sin = np.sin(ang).astype(np.float32).T
    cosT = np.concatenate([cos, cos], axis=0)  # [128, n]
    sinT = np.concatenate([sin, sin], axis=0)
    return np.ascontiguousarray(cosT), np.ascontiguousarray(sinT)


def _build():
    import concourse.bacc as bacc
    import concourse.tile as tile
    import concourse.mybir as mybir
    from contextlib import ExitStack

    f32 = mybir.dt.float32
    f32r = mybir.dt.float32r
    bf16 = mybir.dt.bfloat16
    Exp = mybir.ActivationFunctionType.Exp

    nc = bacc.Bacc("TRN2", target_bir_lowering=False, debug=False)

    xT = nc.dram_tensor("xT", [DIM, N], bf16, kind="ExternalInput")[:]
    wq = nc.dram_tensor("wq", [DIM, GCOLS], bf16, kind="ExternalInput")[:]
    wk = nc.dram_tensor("wk", [DIM, GCOLS], bf16, kind="ExternalInput")[:]
    perm = nc.dram_tensor("perm", [P, P], bf16, kind="ExternalInput")[:]
    wv = nc.dram_tensor("wv", [DIM, GCOLS], bf16, kind="ExternalInput")[:]
    wo = nc.dram_tensor("wo", [GCOLS, DIM], f32r, kind="ExternalInput")[:]
    cosT = nc.dram_tensor("cosT", [P, N], f32, kind="ExternalInput")[:]
    sinT = nc.dram_tensor("sinT", [P, N], f32, kind="ExternalInput")[:]
    out = nc.dram_tensor("out", [N, DIM], f32, kind="ExternalOutput")[:]

    with tile.TileContext(nc) as tc, ExitStack() as ctx:
        pools = {}
        for name, bufs in (("persist", 1), ("xtp", 1), ("wvp", 1), ("trig", 1),
                           ("wst", 1), ("ropetmp", 2), ("wop", 1), ("ep", 26),
                           ("nrm", 3), ("outp", 3)):
            pools[name] = ctx.enter_context(tc.tile_pool(name=name, bufs=bufs))
        psA = ctx.enter_context(tc.tile_pool(name="psA", bufs=2, space="PSUM"))
        psB = ctx.enter_context(tc.tile_pool(name="psB", bufs=2, space="PSUM"))
        psC = ctx.enter_context(tc.tile_pool(name="psC", bufs=2, space="PSUM"))

        persist = pools["persist"]
        qt_sb = persist.tile([P, 2, N], bf16, tag="qt")
        kt_sb = persist.tile([P, 2, N], bf16, tag="kt")
        v_sb = persist.tile([P, NKT, 4, DH + 1], bf16, tag="v")  # [d(64) | ones]
        ot_sb = persist.tile([P, 2, N], f32r, tag="ot")

        # ---- input DMAs
        xt_sb = pools["xtp"].tile([P, KT, N], bf16, tag="xt")
        nc.sync.dma_start(xt_sb[:, 0, :], xT[0:P, :])
        w_sb = {}
        for nm, dr in (("wq", wq), ("wk", wk)):
            w_sb[nm] = pools["wst"].tile([P, KT, GCOLS], bf16, tag=nm, name=nm)
            nc.sync.dma_start(w_sb[nm], dr.rearrange("(ko p) c -> p ko c", p=P))
        for k in range(1, KT):
            nc.sync.dma_start(xt_sb[:, k, :], xT[k * P:(k + 1) * P, :])
        perm_sb = pools["wst"].tile([P, P], bf16, tag="perm")
        nc.sync.dma_start(perm_sb, perm)
        cos_sb = pools["trig"].tile([P, N], f32, tag="cos")
        sin_sb = pools["trig"].tile([P, N], f32, tag="sin")
        nc.sync.dma_start(cos_sb, cosT)
        nc.sync.dma_start(sin_sb, sinT)
        wv_sb = pools["wvp"].tile([P, KT, GCOLS], bf16, tag="wv")
        nc.sync.dma_start(wv_sb, wv.rearrange("(ko p) c -> p ko c", p=P))
        wo_sb = pools["wop"].tile([P, 2, DIM], f32r, tag="wo")
        nc.sync.dma_start(wo_sb, wo.rearrange("(ko p) c -> p ko c", p=P))

        def proj_qk(dst, wname, c, m):
            """Project + RoPE one (nq chunk, head-pair) of Q or K.  The
            rotate-half twin comes from a single 128x128 pair-swap matmul on
            the projection output instead of a second 8-matmul projection."""
            cs = slice(c * NQC, (c + 1) * NQC)
            ps_q = psA.tile([P, NQC], f32, tag="ps_proj")
            for k in range(KT):
                nc.tensor.matmul(ps_q, w_sb[wname][:, k, m * P:(m + 1) * P],
                                 xt_sb[:, k, cs],
                                 start=(k == 0), stop=(k == KT - 1))
            yield
            qtmp = pools["ropetmp"].tile([P, NQC], bf16, tag="qtmp")
            nc.vector.tensor_copy(out=qtmp, in_=ps_q)
            ps_s = psA.tile([P, NQC], f32, tag="ps_proj")
            nc.tensor.matmul(ps_s, perm_sb, qtmp, start=True, stop=True)
            t1 = pools["ropetmp"].tile([P, NQC], f32, tag="rt1")
            t2 = pools["ropetmp"].tile([P, NQC], f32, tag="rt2")
            nc.vector.tensor_mul(t1, ps_q, cos_sb[:, cs])
            nc.vector.tensor_mul(t2, ps_s, sin_sb[:, cs])
            nc.gpsimd.tensor_add(dst[:, m, cs], t1, t2)
            yield

        def run_gen(g):
            for _ in g:
                pass

        def proj_v(t):
            ps_v_full = psA.tile([P, NQC], f32, tag="ps_proj")
            ps_v = ps_v_full[:, :GCOLS]
            for k in range(KT):
                nc.tensor.matmul(ps_v, xt_sb[:, k, t * P:(t + 1) * P], wv_sb[:, k, :],
                                 start=(k == 0), stop=(k == KT - 1))
            yield
            nc.vector.tensor_copy(
                out=v_sb[:, t, :, 0:DH],
                in_=ps_v.rearrange("p (h d) -> p h d", d=DH))
            yield

        def wo_chunk(c):
            """Output projection for nq chunk c: out[cs, :] = Ot[:, :, cs].T @ Wo."""
            for sub in range(NQC // P):
                nt = c * (NQC // P) + sub
                for oc in range(2):
                    ps_w = psA.tile([P, NQC], f32, tag="ps_proj")
                    for k in range(2):
                        nc.tensor.matmul(ps_w, ot_sb[:, k, nt * P:(nt + 1) * P],
                                         wo_sb[:, k, oc * NQC:(oc + 1) * NQC],
                                         start=(k == 0), stop=(k == 1))
                    o_t = pools["outp"].tile([P, NQC], f32, tag="o")
                    nc.vector.tensor_copy(out=o_t, in_=ps_w)
                    nc.sync.dma_start(
                        out[nt * P:(nt + 1) * P, oc * NQC:(oc + 1) * NQC], o_t)
                    yield

        def normalize(ps_o, hp, idx, cs):
            """ot[d, nq] = O'[d, nq] / den[nq] for head (2*hp + idx).
            Evacuates the psum bank fast (recip + raw copy), then divides
            in SBUF off the psum-slot critical path."""
            dst = ot_sb[idx * DH:(idx + 1) * DH, hp, cs]
            rec = pools["nrm"].tile([P, NQC], f32, tag="rec")
            bc = pools["nrm"].tile([P, NQC], f32, tag="bc")
            nc.vector.tensor_copy(out=rec[0:1, :], in_=ps_o[DH:DH + 1, :])
            nc.vector.tensor_copy(out=dst, in_=ps_o[0:DH, :])
            rec2 = pools["nrm"].tile([P, NQC], f32, tag="rec2")
            nc.vector.reciprocal_approx_fast(rec2[0:1, :], rec[0:1, :])
            nc.gpsimd.partition_broadcast(bc, rec2[0:1, :])
            nc.vector.tensor_mul(dst, dst, bc[idx * DH:(idx + 1) * DH, :])

        # ---- preamble: just enough to unblock attention stage (c=0, hp=0)
        with nc.named_scope("preamble"):
            run_gen(proj_qk(qt_sb, "wq", 0, 0))
            run_gen(proj_qk(kt_sb, "wk", 0, 0))
            ones_sb = pools["wvp"].tile([P, 1], f32, tag="ones")
            nc.vector.memset(ones_sb, 1.0)
            nc.vector.tensor_copy(
                out=v_sb[:, :, :, DH],
                in_=ones_sb[:, 0:1].to_broadcast((P, NKT, 4)))
            run_gen(proj_v(0))

        # Remaining projection / output work is spread across the attention
        # stages as small quanta: each entry is (deadline_step, generator).
        # A quantum is emitted no later than its deadline, and one extra
        # quantum per t-step drains the queue early so stage boundaries don't
        # pile up low-priority PE work.
        filler_q = []  # list of (deadline, gen), in emission (deadline) order

        def g_step(si, t):
            return si * NKT + t

        for cc in range(1, NCH):  # kt chunk cc before stage-0 step 4*cc
            for m in range(2):
                filler_q.append((g_step(0, 4 * cc - 2), 0, proj_qk(kt_sb, "wk", cc, m)))
        for t in range(1, NKT):  # v tile t before stage-0 step t
            filler_q.append((g_step(0, t - 1), 0, proj_v(t)))
        # second head-pair of chunk 0, needed from stage (0, 1)
        filler_q.append((g_step(0, 13), 0, proj_qk(qt_sb, "wq", 0, 1)))
        filler_q.append((g_step(0, 11), 0, proj_qk(kt_sb, "wk", 0, 1)))
        for cc in range(1, NCH):  # qt chunk cc before stage 2*cc, spread over 2 stages
            filler_q.append((g_step(2 * cc - 2, 10), 0, proj_qk(qt_sb, "wq", cc, 0)))
            filler_q.append((g_step(2 * cc - 1, 10), 0, proj_qk(qt_sb, "wq", cc, 1)))
        for cc in range(NCH - 1):  # wo chunk cc after stage 2*cc+2 normalizes
            filler_q.append((g_step(2 * cc + 3, 12), g_step(2 * cc + 3, 0), wo_chunk(cc)))

        filler_q.sort(key=lambda e: e[0])

        def emit_fillers(g, extra=1):
            """Emit all overdue quanta plus `extra` opportunistic ones."""
            budget = extra
            while filler_q:
                deadline, earliest, gen = filler_q[0]
                overdue = deadline <= g
                if not overdue and (budget <= 0 or g < earliest):
                    break
                try:
                    next(gen)
                    if not overdue:
                        budget -= 1
                except StopIteration:
                    filler_q.pop(0)

        # ---- attention stages, software-pipelined: head B's PV matmuls for
        # stage i run inside stage i+1's t-loop (reading the cached E tiles),
        # so the PE always has work and ACT (exp) never starves.
        stages = [(c, hp) for c in range(NCH) for hp in range(2)]
        prev = None  # (e_tiles, hp, cs) of the previous stage
        for si, (c, hp) in enumerate(stages):
            cs = slice(c * NQC, (c + 1) * NQC)
            with nc.named_scope(f"attn_c{c}_h{hp}"):
                ps_oA = psC.tile([P, NQC], f32, tag="ps_o")
                if prev is not None:
                    ps_oB_prev = psC.tile([P, NQC], f32, tag="ps_o")
                e_ts = []
                for t in range(NKT):
                    ts_ = slice(t * P, (t + 1) * P)
                    ps_t = psB.tile([P, 2, NQC], f32, tag="ps_t")
                    # S^T tiles for both heads; row groups 0-1 / 2-3 run
                    # concurrently in the PE array.
                    nc.tensor.matmul(ps_t[:, 0, :], kt_sb[0:DH, hp, ts_],
                                     qt_sb[0:DH, hp, cs], start=True, stop=True)
                    nc.tensor.matmul(ps_t[:, 1, :], kt_sb[DH:P, hp, ts_],
                                     qt_sb[DH:P, hp, cs], start=True, stop=True)
                    e_t = pools["ep"].tile([P, 2, NQC], bf16, tag="e")
                    nc.scalar.activation(e_t, ps_t, Exp, scale=0.125)
                    e_ts.append(e_t)
                    nc.tensor.matmul(ps_oA[0:DH + 1, :], v_sb[:, t, 2 * hp, :],
                                     e_t[:, 0, :],
                                     start=(t == 0), stop=(t == NKT - 1))
                    if prev is not None:
                        pe_ts, phf, pcs = prev
                        nc.tensor.matmul(ps_oB_prev[0:DH + 1, :],
                                         v_sb[:, t, 2 * phf + 1, :],
                                         pe_ts[t][:, 1, :],
                                         start=(t == 0), stop=(t == NKT - 1))
                    emit_fillers(g_step(si, t))
                if prev is not None:
                    pe_ts, phf, pcs = prev
                    normalize(ps_oB_prev, phf, 1, pcs)
                normalize(ps_oA, hp, 0, cs)
            prev = (e_ts, hp, cs)

        # epilogue: drain head B of the last stage, then the last Wo chunk
        pe_ts, phf, pcs = prev
        with nc.named_scope("attn_tail"):
            ps_oB_last = psC.tile([P, NQC], f32, tag="ps_o")
            for t in range(NKT):
                nc.tensor.matmul(ps_oB_last[0:DH + 1, :], v_sb[:, t, 2 * phf + 1, :],
                                 pe_ts[t][:, 1, :],
                                 start=(t == 0), stop=(t == NKT - 1))
                emit_fillers(g_step(8, t), extra=0)
            normalize(ps_oB_last, phf, 1, pcs)
        while filler_q:
            emit_fillers(10 ** 9)
        run_gen(wo_chunk(NCH - 1))
    nc.compile()
    return nc


def _prep_inputs(x, Wq, Wkv, Wo):
    """Host-side sharding: returns in_maps for the 8 cores."""
    import ml_dtypes
    bf16 = ml_dtypes.bfloat16
    cosT, sinT = _rope_tables()
    # pair-swap matrix: sq = M @ q with M[2i, 2i+1] = -1, M[2i+1, 2i] = 1;
    # matmul computes lhsT.T @ rhs so pass M.T
    M = np.zeros((P, P), dtype=np.float32)
    for i in range(0, P, 2):
        M[i, i + 1] = -1.0
        M[i + 1, i] = 1.0
    permT = np.ascontiguousarray(M.T).astype(bf16)
    xTs = [np.ascontiguousarray(x[b].T).astype(bf16) for b in range(B)]
    in_maps = []
    for core in range(8):
        b, g = divmod(core, 4)
        cols = slice(g * GCOLS, (g + 1) * GCOLS)
        wq_g = np.ascontiguousarray(Wq[:, cols])
        wk_g = np.ascontiguousarray(Wkv[:, g * GCOLS:(g + 1) * GCOLS])
        wv_g = np.ascontiguousarray(Wkv[:, DIM + g * GCOLS:DIM + (g + 1) * GCOLS])
        wo_g = np.ascontiguousarray(Wo[cols, :])
        in_maps.append({
            "xT": xTs[b],
            "wq": wq_g.astype(bf16), "wk": wk_g.astype(bf16),
            "wv": wv_g.astype(bf16), "wo": wo_g,
            "cosT": cosT, "sinT": sinT, "perm": permT,
        })
    return in_maps


def kernel(x, Wq, Wkv, Wo, bo):
    from concourse.bass_utils import run_bass_kernel_spmd

    x = np.asarray(x, dtype=np.float32)
    Wq = np.asarray(Wq, dtype=np.float32)
    Wkv = np.asarray(Wkv, dtype=np.float32)
    Wo = np.asarray(Wo, dtype=np.float32)
    bo = np.asarray(bo, dtype=np.float32)

    if "nc" not in _cache:
        _cache["nc"] = _build()
    nc = _cache["nc"]

    in_maps = _prep_inputs(x, Wq, Wkv, Wo)
    res = run_bass_kernel_spmd(nc, in_maps, core_ids=list(range(8)))
    _cache["last_results"] = res

    full = np.zeros((B, N, DIM), dtype=np.float32)
    for core in range(8):
        b = core // 4
        full[b] += res.results[core]["out"]
    full += bo[None, None, :]
    return full


# revision 35
# speedup vs baseline: 1.0283x; 1.0283x over previous
"""Multi-head attention (b=2, n=2048, 16 heads x 64, RoPE) on 8 TRN2 NeuronCores.

Sharding: core = 4*b + g handles batch b (0..1) and head-group g (0..3, i.e.
heads 4g..4g+3).  Each core computes its partial output projection
out_partial[b] = O_g @ Wo[256g:256g+256, :]; the host sums the 4 partials per
batch and adds the bias.

Device layout (per core, everything transposed so the contraction dim sits on
SBUF partitions):
  xT   [1024, 2048]  x[b].T  bf16                  (host pre-transposed)
  wq/wqs/wk/wks/wv [1024, 256] bf16, wo [256, 1024] f32r
  cosT/sinT [128, 2048] f32                        (RoPE tables, head-pair rows)
Pipeline: QK projections (+RoPE-swapped twins, bf16) -> RoPE combine on DVE
(fp32 psum x fp32 trig -> bf16) -> V projection -> attention per nq-chunk and
head-pair: S^T = K Q^T on PE (row-group concurrent head pairs, bf16), exp on
ACT (fp32 psum -> bf16), O'^T = [V|1]^T P^T (bf16 in, fp32 accum; ones column
yields softmax denominators) -> normalize (fp32) -> f32r output projection,
emitted per nq chunk so it overlaps the next chunk's attention.
"""

import numpy as np

HEADS = 16
DH = 64
THETA = 10000.0
B, N, DIM = 2, 2048, 1024
GCOLS = 4 * DH  # 256 columns per head-group
P = 128
NQC = 512  # nq chunk (psum bank)
NCH = N // NQC  # 4
KT = DIM // P  # 8 contraction subtiles for projections
NKT = N // P  # 16 nk tiles

_cache = {}


def _rope_tables():
    """cosT/sinT [128, 2048] in transposed (d, n) layout, head-pair rows.
    Replicates reference fp32 arithmetic."""
    d = np.float32(DH)
    inv_freq = np.float32(1.0) / (
        np.float32(THETA) ** (np.arange(0, DH, 2, dtype=np.float32) / d)
    )  # [32]
    ang = np.arange(N, dtype=np.float32)[:, None] * inv_freq[None, :]  # [n, 32] fp32
    ang = np.repeat(ang, 2, axis=-1)  # [n, 64]
    cos = np.cos(ang).astype(np.float32).T  # [64, n]
    sin = np.sin(ang).astype(np.float32).T
    cosT = np.concatenate([cos, cos], axis=0)  # [128, n]
    sinT = np.concatenate([sin, sin], axis=0)
    return np.ascontiguousarray(cosT), np.ascontiguousarray(sinT)


def _build():
    import concourse.bacc as bacc
    import concourse.tile as tile
    import concourse.mybir as mybir
    from contextlib import ExitStack

    f32 = mybir.dt.float32
    f32r = mybir.dt.float32r
    bf16 = mybir.dt.bfloat16
    Exp = mybir.ActivationFunctionType.Exp

    nc = bacc.Bacc("TRN2", target_bir_lowering=False, debug=False)

    xT = nc.dram_tensor("xT", [DIM, N], bf16, kind="ExternalInput")[:]
    wq = nc.dram_tensor("wq", [DIM, GCOLS], bf16, kind="ExternalInput")[:]
    wk = nc.dram_tensor("wk", [DIM, GCOLS], bf16, kind="ExternalInput")[:]
    perm = nc.dram_tensor("perm", [P, P], bf16, kind="ExternalInput")[:]
    wv = nc.dram_tensor("wv", [DIM, GCOLS], bf16, kind="ExternalInput")[:]
    wo = nc.dram_tensor("wo", [GCOLS, DIM], f32r, kind="ExternalInput")[:]
    cosT = nc.dram_tensor("cosT", [P, N], f32, kind="ExternalInput")[:]
    sinT = nc.dram_tensor("sinT", [P, N], f32, kind="ExternalInput")[:]
    out = nc.dram_tensor("out", [N, DIM], f32, kind="ExternalOutput")[:]

    with tile.TileContext(nc) as tc, ExitStack() as ctx:
        pools = {}
        for name, bufs in (("persist", 1), ("xtp", 1), ("wvp", 1), ("trig", 1),
                           ("wst", 1), ("ropetmp", 2), ("wop", 1), ("ep", 24),
                           ("nrm", 3), ("outp", 3)):
            pools[name] = ctx.enter_context(tc.tile_pool(name=name, bufs=bufs))
        psA = ctx.enter_context(tc.tile_pool(name="psA", bufs=2, space="PSUM"))
        psB = ctx.enter_context(tc.tile_pool(name="psB", bufs=2, space="PSUM"))
        psC = ctx.enter_context(tc.tile_pool(name="psC", bufs=2, space="PSUM"))

        persist = pools["persist"]
        qt_sb = persist.tile([P, 2, N], bf16, tag="qt")
        kt_sb = persist.tile([P, 2, N], bf16, tag="kt")
        v_sb = persist.tile([P, NKT, 4, DH + 1], bf16, tag="v")  # [d(64) | ones]
        ot_sb = persist.tile([P, 2, N], f32r, tag="ot")

        # ---- input DMAs
        xt_sb = pools["xtp"].tile([P, KT, N], bf16, tag="xt")
        for k in range(KT):
            nc.sync.dma_start(xt_sb[:, k, :], xT[k * P:(k + 1) * P, :])
        w_sb = {}
        for nm, dr in (("wq", wq), ("wk", wk)):
            w_sb[nm] = pools["wst"].tile([P, KT, GCOLS], bf16, tag=nm, name=nm)
            nc.sync.dma_start(w_sb[nm], dr.rearrange("(ko p) c -> p ko c", p=P))
        perm_sb = pools["wst"].tile([P, P], bf16, tag="perm")
        nc.sync.dma_start(perm_sb, perm)
        cos_sb = pools["trig"].tile([P, N], f32, tag="cos")
        sin_sb = pools["trig"].tile([P, N], f32, tag="sin")
        nc.sync.dma_start(cos_sb, cosT)
        nc.sync.dma_start(sin_sb, sinT)
        wv_sb = pools["wvp"].tile([P, KT, GCOLS], bf16, tag="wv")
        nc.sync.dma_start(wv_sb, wv.rearrange("(ko p) c -> p ko c", p=P))
        wo_sb = pools["wop"].tile([P, 2, DIM], f32r, tag="wo")
        nc.sync.dma_start(wo_sb, wo.rearrange("(ko p) c -> p ko c", p=P))

        def proj_qk(dst, wname, c, m):
            """Project + RoPE one (nq chunk, head-pair) of Q or K.  The
            rotate-half twin comes from a single 128x128 pair-swap matmul on
            the projection output instead of a second 8-matmul projection."""
            cs = slice(c * NQC, (c + 1) * NQC)
            ps_q = psA.tile([P, NQC], f32, tag="ps_proj")
            for k in range(KT):
                nc.tensor.matmul(ps_q, w_sb[wname][:, k, m * P:(m + 1) * P],
                                 xt_sb[:, k, cs],
                                 start=(k == 0), stop=(k == KT - 1))
            yield
            qtmp = pools["ropetmp"].tile([P, NQC], bf16, tag="qtmp")
            nc.vector.tensor_copy(out=qtmp, in_=ps_q)
            ps_s = psA.tile([P, NQC], f32, tag="ps_proj")
            nc.tensor.matmul(ps_s, perm_sb, qtmp, start=True, stop=True)
            t1 = pools["ropetmp"].tile([P, NQC], f32, tag="rt1")
            t2 = pools["ropetmp"].tile([P, NQC], f32, tag="rt2")
            nc.vector.tensor_mul(t1, ps_q, cos_sb[:, cs])
            nc.vector.tensor_mul(t2, ps_s, sin_sb[:, cs])
            nc.gpsimd.tensor_add(dst[:, m, cs], t1, t2)
            yield

        def run_gen(g):
            for _ in g:
                pass

        def proj_v(t):
            ps_v_full = psA.tile([P, NQC], f32, tag="ps_proj")
            ps_v = ps_v_full[:, :GCOLS]
            for k in range(KT):
                nc.tensor.matmul(ps_v, xt_sb[:, k, t * P:(t + 1) * P], wv_sb[:, k, :],
                                 start=(k == 0), stop=(k == KT - 1))
            yield
            nc.vector.tensor_copy(
                out=v_sb[:, t, :, 0:DH],
                in_=ps_v.rearrange("p (h d) -> p h d", d=DH))
            yield

        def wo_chunk(c):
            """Output projection for nq chunk c: out[cs, :] = Ot[:, :, cs].T @ Wo."""
            for sub in range(NQC // P):
                nt = c * (NQC // P) + sub
                for oc in range(2):
                    ps_w = psA.tile([P, NQC], f32, tag="ps_proj")
                    for k in range(2):
                        nc.tensor.matmul(ps_w, ot_sb[:, k, nt * P:(nt + 1) * P],
                                         wo_sb[:, k, oc * NQC:(oc + 1) * NQC],
                                         start=(k == 0), stop=(k == 1))
                    o_t = pools["outp"].tile([P, NQC], f32, tag="o")
                    nc.vector.tensor_copy(out=o_t, in_=ps_w)
                    nc.sync.dma_start(
                        out[nt * P:(nt + 1) * P, oc * NQC:(oc + 1) * NQC], o_t)
                    yield

        def normalize(ps_o, hp, idx, cs):
            """ot[d, nq] = O'[d, nq] / den[nq] for head (2*hp + idx).
            Evacuates the psum bank fast (recip + raw copy), then divides
            in SBUF off the psum-slot critical path."""
            dst = ot_sb[idx * DH:(idx + 1) * DH, hp, cs]
            rec = pools["nrm"].tile([P, NQC], f32, tag="rec")
            bc = pools["nrm"].tile([P, NQC], f32, tag="bc")
            nc.vector.tensor_copy(out=rec[0:1, :], in_=ps_o[DH:DH + 1, :])
            nc.vector.tensor_copy(out=dst, in_=ps_o[0:DH, :])
            rec2 = pools["nrm"].tile([P, NQC], f32, tag="rec2")
            nc.vector.reciprocal_approx_fast(rec2[0:1, :], rec[0:1, :])
            nc.gpsimd.partition_broadcast(bc, rec2[0:1, :])
            nc.vector.tensor_mul(dst, dst, bc[idx * DH:(idx + 1) * DH, :])

        # ---- preamble: just enough to unblock attention stage (c=0, hp=0)
        with nc.named_scope("preamble"):
            run_gen(proj_qk(qt_sb, "wq", 0, 0))
            run_gen(proj_qk(kt_sb, "wk", 0, 0))
            ones_sb = pools["wvp"].tile([P, 1], f32, tag="ones")
            nc.vector.memset(ones_sb, 1.0)
            nc.vector.tensor_copy(
                out=v_sb[:, :, :, DH],
                in_=ones_sb[:, 0:1].to_broadcast((P, NKT, 4)))
            run_gen(proj_v(0))

        # Remaining projection / output work is spread across the attention
        # stages as small quanta: each entry is (deadline_step, generator).
        # A quantum is emitted no later than its deadline, and one extra
        # quantum per t-step drains the queue early so stage boundaries don't
        # pile up low-priority PE work.
        filler_q = []  # list of (deadline, gen), in emission (deadline) order

        def g_step(si, t):
            return si * NKT + t

        for cc in range(1, NCH):  # kt chunk cc before stage-0 step 4*cc
            for m in range(2):
                filler_q.append((g_step(0, 4 * cc - 2), 0, proj_qk(kt_sb, "wk", cc, m)))
        for t in range(1, NKT):  # v tile t before stage-0 step t
            filler_q.append((g_step(0, t - 1), 0, proj_v(t)))
        # second head-pair of chunk 0, needed from stage (0, 1)
        filler_q.append((g_step(0, 13), 0, proj_qk(qt_sb, "wq", 0, 1)))
        filler_q.append((g_step(0, 11), 0, proj_qk(kt_sb, "wk", 0, 1)))
        for cc in range(1, NCH):  # qt chunk cc before stage 2*cc
            for m in range(2):
                filler_q.append((g_step(2 * cc - 1, 10), 0, proj_qk(qt_sb, "wq", cc, m)))
        for cc in range(NCH - 1):  # wo chunk cc after stage 2*cc+2 normalizes
            filler_q.append((g_step(2 * cc + 3, 12), g_step(2 * cc + 3, 0), wo_chunk(cc)))

        filler_q.sort(key=lambda e: e[0])

        def emit_fillers(g, extra=1):
            """Emit all overdue quanta plus `extra` opportunistic ones."""
            budget = extra
            while filler_q:
                deadline, earliest, gen = filler_q[0]
                overdue = deadline <= g
                if not overdue and (budget <= 0 or g < earliest):
                    break
                try:
                    next(gen)
                    if not overdue:
                        budget -= 1
                except StopIteration:
                    filler_q.pop(0)

        # ---- attention stages, software-pipelined: head B's PV matmuls for
        # stage i run inside stage i+1's t-loop (reading the cached E tiles),
        # so the PE always has work and ACT (exp) never starves.
        stages = [(c, hp) for c in range(NCH) for hp in range(2)]
        prev = None  # (e_tiles, hp, cs) of the previous stage
        for si, (c, hp) in enumerate(stages):
            cs = slice(c * NQC, (c + 1) * NQC)
            with nc.named_scope(f"attn_c{c}_h{hp}"):
                ps_oA = psC.tile([P, NQC], f32, tag="ps_o")
                if prev is not None:
                    ps_oB_prev = psC.tile([P, NQC], f32, tag="ps_o")
                e_ts = []
                for t in range(NKT):
                    ts_ = slice(t * P, (t + 1) * P)
                    ps_t = psB.tile([P, 2, NQC], f32, tag="ps_t")
                    # S^T tiles for both heads; row groups 0-1 / 2-3 run
                    # concurrently in the PE array.
                    nc.tensor.matmul(ps_t[:, 0, :], kt_sb[0:DH, hp, ts_],
                                     qt_sb[0:DH, hp, cs], start=True, stop=True)
                    nc.tensor.matmul(ps_t[:, 1, :], kt_sb[DH:P, hp, ts_],
                                     qt_sb[DH:P, hp, cs], start=True, stop=True)
                    e_t = pools["ep"].tile([P, 2, NQC], bf16, tag="e")
                    nc.scalar.activation(e_t, ps_t, Exp, scale=0.125)
                    e_ts.append(e_t)
                    nc.tensor.matmul(ps_oA[0:DH + 1, :], v_sb[:, t, 2 * hp, :],
                                     e_t[:, 0, :],
                                     start=(t == 0), stop=(t == NKT - 1))
                    if prev is not None:
                        pe_ts, phf, pcs = prev
                        nc.tensor.matmul(ps_oB_prev[0:DH + 1, :],
                                         v_sb[:, t, 2 * phf + 1, :],
                                         pe_ts[t][:, 1, :],
                                         start=(t == 0), stop=(t == NKT - 1))
                    emit_fillers(g_step(si, t))
                if prev is not None:
                    pe_ts, phf, pcs = prev
                    normalize(ps_oB_prev, phf, 1, pcs)
                normalize(ps_oA, hp, 0, cs)
            prev = (e_ts, hp, cs)

        # epilogue: drain head B of the last stage, then the last Wo chunk
        pe_ts, phf, pcs = prev
        with nc.named_scope("attn_tail"):
            ps_oB_last = psC.tile([P, NQC], f32, tag="ps_o")
            for t in range(NKT):
                nc.tensor.matmul(ps_oB_last[0:DH + 1, :], v_sb[:, t, 2 * phf + 1, :],
                                 pe_ts[t][:, 1, :],
                                 start=(t == 0), stop=(t == NKT - 1))
                emit_fillers(g_step(8, t), extra=0)
            normalize(ps_oB_last, phf, 1, pcs)
        while filler_q:
            emit_fillers(10 ** 9)
        run_gen(wo_chunk(NCH - 1))
    nc.compile()
    return nc


def _prep_inputs(x, Wq, Wkv, Wo):
    """Host-side sharding: returns in_maps for the 8 cores."""
    import ml_dtypes
    bf16 = ml_dtypes.bfloat16
    cosT, sinT = _rope_tables()
    # pair-swap matrix: sq = M @ q with M[2i, 2i+1] = -1, M[2i+1, 2i] = 1;
    # matmul computes lhsT.T @ rhs so pass M.T
    M = np.zeros((P, P), dtype=np.float32)
    for i in range(0, P, 2):
        M[i, i + 1] = -1.0
        M[i + 1, i] = 1.0
    permT = np.ascontiguousarray(M.T).astype(bf16)
    xTs = [np.ascontiguousarray(x[b].T).astype(bf16) for b in range(B)]
    in_maps = []
    for core in range(8):
        b, g = divmod(core, 4)
        cols = slice(g * GCOLS, (g + 1) * GCOLS)
        wq_g = np.ascontiguousarray(Wq[:, cols])
        wk_g = np.ascontiguousarray(Wkv[:, g * GCOLS:(g + 1) * GCOLS])
        wv_g = np.ascontiguousarray(Wkv[:, DIM + g * GCOLS:DIM + (g + 1) * GCOLS])
        wo_g = np.ascontiguousarray(Wo[cols, :])
        in_maps.append({
            "xT": xTs[b],
            "wq": wq_g.astype(bf16), "wk": wk_g.astype(bf16),
            "wv": wv_g.astype(bf16), "wo": wo_g,
            "cosT": cosT, "sinT": sinT, "perm": permT,
        })
    return in_maps


def kernel(x, Wq, Wkv, Wo, bo):
    from concourse.bass_utils import run_bass_kernel_spmd

    x = np.asarray(x, dtype=np.float32)
    Wq = np.asarray(Wq, dtype=np.float32)
    Wkv = np.asarray(Wkv, dtype=np.float32)
    Wo = np.asarray(Wo, dtype=np.float32)
    bo = np.asarray(bo, dtype=np.float32)

    if "nc" not in _cache:
        _cache["nc"] = _build()
    nc = _cache["nc"]

    in_maps = _prep_inputs(x, Wq, Wkv, Wo)
    res = run_bass_kernel_spmd(nc, in_maps, core_ids=list(range(8)))
    _cache["last_results"] = res

    full = np.zeros((B, N, DIM), dtype=np.float32)
    for core in range(8):
        b = core // 4
        full[b] += res.results[core]["out"]
    full += bo[None, None, :]
    return full
